# revision 5
# baseline (speedup 1.0000x reference)
"""CoAttention kernel for Trainium2 (8 NeuronCores, batch-parallel SPMD).

Per batch b:
  pinfo = mean(patient, L) @ W3 + b3
  c_v = query @ W1v + pinfo ; c_s = query @ W1s + pinfo
  v_attn = softmax(tanh(visual @ W2v + c_v) @ vv)  over NV
  s_attn = softmax(tanh(semantic @ W2s + c_s) @ vs) over NS
  ctx = concat(v_attn @ visual, s_attn @ semantic) @ W + bW

Sharding: 32 batches -> 4 per core x 8 cores (data parallel, weights
replicated). All matmuls run in float32r (full-rate PE, ~1e-4 rel err).
Features are transposed on-chip with PE is_transpose into [feat, rows]
tiles that feed the scores matmuls; softmax skips max-subtraction
(scores are O(1)); the 1/sum normalizations are applied to the
attention outputs directly and folded into the final projection for ctx.
"""
import sys
sys.path.insert(0, "/opt/trn_rl_repo")

import numpy as np
import concourse.bass as bass
import concourse.mybir as mybir
import concourse.tile as tile
from concourse import bacc, bass_utils

P = 128
B, NV, NS, L = 32, 2048, 512, 64
FEAT, SEM, HID, ATT, DM = 1024, 512, 512, 512, 512
NCORES = 8
BL = B // NCORES
RCH = 512
NCHV = NV // RCH
KFV = FEAT // P
KFS = SEM // P
MA = ATT // P
KW = (FEAT + SEM) // P

F32R = mybir.dt.float32r
F32 = mybir.dt.float32
AF = mybir.ActivationFunctionType
ALU = mybir.AluOpType
AX = mybir.AxisListType

_CACHE = {}


def _build():
    nc = bacc.Bacc("TRN2", target_bir_lowering=False, debug=False,
                   num_devices=NCORES)
    d_query = nc.dram_tensor("query", [BL, HID], F32R, kind="ExternalInput")
    d_vis = nc.dram_tensor("visual", [BL * NV, FEAT], F32R, kind="ExternalInput")
    d_sem = nc.dram_tensor("semantic", [BL * NS, SEM], F32R, kind="ExternalInput")
    d_pat = nc.dram_tensor("patient", [BL * L, DM], F32R, kind="ExternalInput")
    d_w1v = nc.dram_tensor("W1v", [HID, ATT], F32R, kind="ExternalInput")
    d_w2v = nc.dram_tensor("W2v", [FEAT, ATT], F32R, kind="ExternalInput")
    d_vv = nc.dram_tensor("vv", [ATT], F32R, kind="ExternalInput")
    d_w3 = nc.dram_tensor("W3", [DM, ATT], F32R, kind="ExternalInput")
    d_b3 = nc.dram_tensor("b3", [1, ATT], F32R, kind="ExternalInput")
    d_w1s = nc.dram_tensor("W1s", [HID, ATT], F32R, kind="ExternalInput")
    d_w2s = nc.dram_tensor("W2s", [SEM, ATT], F32R, kind="ExternalInput")
    d_vs = nc.dram_tensor("vs", [ATT], F32R, kind="ExternalInput")
    d_w = nc.dram_tensor("W", [FEAT + SEM, FEAT], F32R, kind="ExternalInput")
    d_bw = nc.dram_tensor("bW", [1, FEAT], F32R, kind="ExternalInput")
    d_ident = nc.dram_tensor("ident", [P, P], F32R, kind="ExternalInput")
    d_ones = nc.dram_tensor("ones_col", [P, 1], F32R, kind="ExternalInput")
    d_ones_row = nc.dram_tensor("ones_row", [1, P], F32R, kind="ExternalInput")

    d_ctx = nc.dram_tensor("ctx", [BL, FEAT], F32R, kind="ExternalOutput")
    d_vattn = nc.dram_tensor("v_attn", [BL, NV], F32R, kind="ExternalOutput")
    d_sattn = nc.dram_tensor("s_attn", [BL, NS], F32R, kind="ExternalOutput")

    with tile.TileContext(nc) as tc:
        with (
            tc.tile_pool(name="const", bufs=1) as const,
            tc.tile_pool(name="fpool", bufs=2) as fpool,
            tc.tile_pool(name="gpool", bufs=2) as gpool,
            tc.tile_pool(name="tpool", bufs=2) as tpool,
            tc.tile_pool(name="setup", bufs=1) as setup,
            tc.tile_pool(name="spool", bufs=1) as spool,
            tc.tile_pool(name="scrp", bufs=1) as scrp,
            tc.tile_pool(name="pt", bufs=2, space="PSUM") as pt,
            tc.tile_pool(name="pm", bufs=4, space="PSUM") as pm,
            tc.tile_pool(name="pss", bufs=1, space="PSUM") as pss,
            tc.tile_pool(name="prr", bufs=1, space="PSUM") as prr,
        ):
            # ---- constants / weights resident in SBUF ----
            ident_r = const.tile([P, P], F32R)
            nc.sync.dma_start(ident_r[:], d_ident.ap())
            ones_col = const.tile([P, 1], F32R)
            nc.sync.dma_start(ones_col[:], d_ones.ap())
            ones_row = const.tile([1, P], F32R)
            nc.sync.dma_start(ones_row[:], d_ones_row.ap())

            def load_ktiles(name, dram, kf, width):
                t = const.tile([P, kf * width], F32R, tag=name)
                for k in range(kf):
                    nc.sync.dma_start(t[:, k * width:(k + 1) * width],
                                      dram.ap()[k * P:(k + 1) * P, :])
                return t

            w2v = load_ktiles("w2v", d_w2v, KFV, ATT)
            w2s = load_ktiles("w2s", d_w2s, KFS, ATT)
            w1v = load_ktiles("w1v", d_w1v, MA, ATT)
            w1s = load_ktiles("w1s", d_w1s, MA, ATT)
            w3t = load_ktiles("w3t", d_w3, MA, ATT)
            vv_sb = const.tile([P, MA], F32R)
            vs_sb = const.tile([P, MA], F32R)
            for m in range(MA):
                nc.sync.dma_start(vv_sb[:, m:m + 1], d_vv.ap()[m * P:(m + 1) * P, None])
                nc.sync.dma_start(vs_sb[:, m:m + 1], d_vs.ap()[m * P:(m + 1) * P, None])
            b3_sb = const.tile([1, ATT], F32R)
            nc.sync.dma_start(b3_sb[:], d_b3.ap())
            bw_sb = const.tile([1, FEAT], F32R)
            nc.sync.dma_start(bw_sb[:], d_bw.ap())
            q_sb = const.tile([1, BL * HID], F32R)
            nc.sync.dma_start(q_sb[:], d_query.ap().rearrange("a b -> (a b)")[None, :])

            # bW replicated to BL rows
            bwrep = const.tile([BL, FEAT], F32)
            for j in range(2):
                bj = pm.tile([BL, 512], F32, tag="pm")
                nc.tensor.matmul(bj[:], ones_row[:, 0:BL],
                                 bw_sb[:, j * 512:(j + 1) * 512],
                                 start=True, stop=True)
                nc.scalar.copy(bwrep[:, j * 512:(j + 1) * 512], bj[:])

            # ---- per-batch setup: c_v / c_s columns [P, MA] ----
            c_both_all = const.tile([P, BL, 2 * MA, 4], F32)
            for b in range(BL):
                patb = setup.tile([L, DM], F32R, tag="patb")
                nc.sync.dma_start(patb[:], d_pat.ap()[b * L:(b + 1) * L, :])
                pm_ps = pss.tile([1, DM], F32, tag="ps")
                nc.tensor.matmul(pm_ps[:], ones_col[0:L, :], patb[:],
                                 start=True, stop=True)
                pmean = setup.tile([1, DM], F32R, tag="pmean")
                nc.scalar.activation(pmean[:], pm_ps[:], AF.Copy, scale=1.0 / L)
                # row -> column via K=1 matmuls (out[:, m] = row_chunk.T)
                colt_ps = pt.tile([P, 2 * MA, 4], F32, tag="pt")
                for m in range(MA):
                    nc.tensor.matmul(colt_ps[:, m, :],
                                     pmean[:, m * P:(m + 1) * P],
                                     ones_row[0:1, 0:4], start=True, stop=True)
                    nc.tensor.matmul(colt_ps[:, MA + m, :],
                                     q_sb[0:1, b * HID + m * P: b * HID + (m + 1) * P],
                                     ones_row[0:1, 0:4], start=True, stop=True)
                cols = setup.tile([P, 2 * MA, 4], F32R, tag="cols")
                nc.scalar.copy(cols[:], colt_ps[:])  # [:, :MA, 0]=pmeanT, [:, MA:, 0]=qT
                for half, w1 in ((0, w1v), (1, w1s)):
                    c_ps = pss.tile([1, ATT], F32, tag="ps")
                    for k in range(MA):
                        nc.tensor.matmul(c_ps[:], cols[:, MA + k, 0:1],
                                         w1[:, k * ATT:(k + 1) * ATT],
                                         start=(k == 0), stop=False)
                    for k in range(MA):
                        nc.tensor.matmul(c_ps[:], cols[:, k, 0:1],
                                         w3t[:, k * ATT:(k + 1) * ATT],
                                         start=False, stop=False)
                    nc.tensor.matmul(c_ps[:], ones_row[0:1, 0:1], b3_sb[:],
                                     start=False, stop=True)
                    c_row = setup.tile([1, ATT], F32R, tag="crow")
                    nc.scalar.copy(c_row[:], c_ps[:])
                    ct_ps = pt.tile([P, MA, 4], F32, tag="pt")
                    for m in range(MA):
                        nc.tensor.matmul(ct_ps[:, m, :],
                                         c_row[:, m * P:(m + 1) * P],
                                         ones_row[0:1, 0:4], start=True, stop=True)
                    nc.scalar.copy(c_both_all[:, b, half * MA:(half + 1) * MA, :],
                                   ct_ps[:])

            # ---- main attention branches ----
            sums_v = const.tile([1, BL * NCHV], F32)
            sums_s = const.tile([1, BL], F32)
            invv_row = const.tile([1, BL], F32)
            invs_row = const.tile([1, BL], F32)
            ccall = const.tile([P, KW, BL], F32)
            ccpv = const.tile([P, KFV, NCHV], F32, tag="ccpv")

            def branch(b, feat_ap, kf, nch, w2, vtile, c_fn, e_tile, sums_fn,
                       ccp_fn):
                for ci in range(nch):
                    fts = []
                    for r in range(RCH // P):
                        ft = fpool.tile([P, kf * P], F32R, tag=f"ft{r}")
                        nc.sync.dma_start(
                            ft[:],
                            feat_ap[ci * RCH + r * P: ci * RCH + (r + 1) * P, :])
                        fts.append(ft)
                    gts = []
                    for f in range(kf):
                        pst = pt.tile([P, RCH], F32R, tag="pt")
                        for r in range(RCH // P):
                            nc.tensor.transpose(pst[:, r * P:(r + 1) * P],
                                                fts[r][:, f * P:(f + 1) * P],
                                                ident_r[:])
                        gt = gpool.tile([P, RCH], F32R, tag=f"g{f}")
                        nc.scalar.copy(gt[:], pst[:])
                        gts.append(gt)
                    tts = []
                    for m in range(MA):
                        psm = pm.tile([P, RCH], F32, tag="pm")
                        for k in range(kf):
                            nc.tensor.matmul(
                                psm[:], w2[:, k * ATT + m * P: k * ATT + (m + 1) * P],
                                gts[k][:], start=(k == 0), stop=(k == kf - 1))
                        tt = tpool.tile([P, RCH], F32R, tag=f"t{m}")
                        nc.scalar.activation(tt[:], psm[:], AF.Tanh,
                                             bias=c_fn(m))
                        tts.append(tt)
                    s_ps = pss.tile([1, RCH], F32, tag="ps")
                    for m in range(MA):
                        nc.tensor.matmul(s_ps[:], vtile[:, m:m + 1], tts[m][:],
                                         start=(m == 0), stop=(m == MA - 1))
                    e_sl = e_tile[:, ci * RCH:(ci + 1) * RCH]
                    nc.scalar.activation(e_sl, s_ps[:], AF.Exp,
                                         accum_out=sums_fn(ci))
                    rep_ps = prr.tile([P, RCH], F32, tag="rep")
                    nc.tensor.matmul(rep_ps[:], ones_row[:], e_sl,
                                     start=True, stop=True)
                    for h in range(kf // 4):
                        scr = scrp.tile([P, 4, RCH], F32, tag="scr")
                        for fi in range(4):
                            nc.vector.tensor_tensor(
                                out=scr[:, fi, :],
                                in0=gts[h * 4 + fi][:].bitcast(F32),
                                in1=rep_ps[:], op=ALU.mult)
                        nc.vector.tensor_reduce(out=ccp_fn(ci, h), in_=scr[:],
                                                axis=AX.X, op=ALU.add)

            for b in range(BL):
                ev = spool.tile([1, NV], F32R, tag="ev")
                branch(b, d_vis.ap()[b * NV:(b + 1) * NV, :], KFV, NCHV, w2v,
                       vv_sb, (lambda m, b=b: c_both_all[:, b, m, 0:1]), ev,
                       lambda ci, b=b: sums_v[:, b * NCHV + ci:b * NCHV + ci + 1],
                       lambda ci, h: ccpv[:, h * 4:(h + 1) * 4, ci])
                nc.vector.tensor_reduce(out=ccall[:, 0:KFV, b], in_=ccpv[:],
                                        axis=AX.X, op=ALU.add)
                sv1 = const.tile([1, BL], F32, tag=f"sv1")
                nc.vector.tensor_reduce(
                    out=sv1[:, b:b + 1],
                    in_=sums_v[:, b * NCHV:(b + 1) * NCHV],
                    axis=AX.X, op=ALU.add)
                nc.vector.reciprocal(invv_row[:, b:b + 1], sv1[:, b:b + 1])
                van = spool.tile([1, NV], F32R, tag="van")
                nc.scalar.activation(van[:], ev.bitcast(F32)[:], AF.Copy,
                                     scale=invv_row[:, b:b + 1])
                nc.gpsimd.dma_start(d_vattn.ap()[b:b + 1, :], van[:])

                es = spool.tile([1, NS], F32R, tag="es")
                branch(b, d_sem.ap()[b * NS:(b + 1) * NS, :], KFS, 1, w2s,
                       vs_sb, (lambda m, b=b: c_both_all[:, b, MA + m, 0:1]), es,
                       lambda ci, b=b: sums_s[:, b:b + 1],
                       lambda ci, h, b=b: ccall[:, KFV:KW, b])
                nc.vector.reciprocal(invs_row[:, b:b + 1], sums_s[:, b:b + 1])
                san = spool.tile([1, NS], F32R, tag="san")
                nc.scalar.activation(san[:], es.bitcast(F32)[:], AF.Copy,
                                     scale=invs_row[:, b:b + 1])
                nc.gpsimd.dma_start(d_sattn.ap()[b:b + 1, :], san[:])

            # ---- final projection ----
            invv_r = const.tile([1, BL], F32R)
            nc.vector.tensor_copy(invv_r[:], invv_row[:])
            invs_r = const.tile([1, BL], F32R)
            nc.vector.tensor_copy(invs_r[:], invs_row[:])
            invc_ps = pt.tile([BL, 8], F32, tag="pt")
            nc.tensor.matmul(invc_ps[:, 0:4], invv_r[:], ones_row[0:1, 0:4],
                             start=True, stop=True)
            nc.tensor.matmul(invc_ps[:, 4:8], invs_r[:], ones_row[0:1, 0:4],
                             start=True, stop=True)
            invc = const.tile([BL, 8], F32)
            nc.scalar.copy(invc[:], invc_ps[:])

            ccr = const.tile([P, KW, BL], F32R)
            nc.vector.tensor_copy(ccr[:], ccall[:])
            out_sb = const.tile([BL, FEAT], F32R)
            tmp_v = const.tile([BL, FEAT], F32, tag="tmpv")
            tmp_s = const.tile([BL, FEAT], F32, tag="tmps")
            for j in range(2):
                sl = slice(j * 512, (j + 1) * 512)
                pv = pt.tile([BL, 512], F32, tag="pt")
                ps2 = pt.tile([BL, 512], F32, tag="pt")
                for k in range(KFV):
                    wk = fpool.tile([P, FEAT], F32R, tag="ft0")
                    nc.sync.dma_start(wk[:], d_w.ap()[k * P:(k + 1) * P, :])
                    nc.tensor.matmul(pv[:], ccr[:, k, :], wk[:, sl],
                                     start=(k == 0), stop=(k == KFV - 1))
                for ki, k in enumerate(range(KFV, KW)):
                    wk = fpool.tile([P, FEAT], F32R, tag="ft1")
                    nc.sync.dma_start(wk[:], d_w.ap()[k * P:(k + 1) * P, :])
                    nc.tensor.matmul(ps2[:], ccr[:, k, :], wk[:, sl],
                                     start=(ki == 0), stop=(k == KW - 1))
                nc.scalar.activation(tmp_v[:, sl], pv[:], AF.Copy,
                                     scale=invc[:, 0:1])
                nc.vector.tensor_scalar_mul(tmp_s[:, sl], ps2[:], invc[:, 4:5])
            nc.vector.tensor_add(tmp_v[:], tmp_v[:], tmp_s[:])
            nc.vector.tensor_add(out_sb.bitcast(F32)[:], tmp_v[:], bwrep[:])
            nc.gpsimd.dma_start(d_ctx.ap(), out_sb[:])
    nc.compile()
    return nc


def kernel(**inputs):
    if "nc" not in _CACHE:
        _CACHE["nc"] = _build()
    nc = _CACHE["nc"]

    q = np.asarray(inputs["query"], dtype=np.float32).reshape(B, HID)
    vis = np.ascontiguousarray(np.asarray(inputs["visual_features"], np.float32))
    sem = np.ascontiguousarray(np.asarray(inputs["semantic_features"], np.float32))
    pat = np.ascontiguousarray(np.asarray(inputs["patient_info"], np.float32))
    shared = {
        "W1v": np.ascontiguousarray(np.asarray(inputs["W1v"], np.float32)),
        "W2v": np.ascontiguousarray(np.asarray(inputs["W2v"], np.float32)),
        "vv": np.asarray(inputs["vv"], np.float32).reshape(ATT),
        "W3": np.ascontiguousarray(np.asarray(inputs["W3"], np.float32)),
        "b3": np.asarray(inputs["b3"], np.float32).reshape(1, ATT),
        "W1s": np.ascontiguousarray(np.asarray(inputs["W1s"], np.float32)),
        "W2s": np.ascontiguousarray(np.asarray(inputs["W2s"], np.float32)),
        "vs": np.asarray(inputs["vs"], np.float32).reshape(ATT),
        "W": np.ascontiguousarray(np.asarray(inputs["W"], np.float32)),
        "bW": np.asarray(inputs["bW"], np.float32).reshape(1, FEAT),
        "ident": np.eye(P, dtype=np.float32),
        "ones_col": np.ones((P, 1), dtype=np.float32),
        "ones_row": np.ones((1, P), dtype=np.float32),
    }
    in_maps = []
    for c in range(NCORES):
        b0 = c * BL
        in_maps.append(dict(
            query=q[b0:b0 + BL],
            visual=vis[b0:b0 + BL].reshape(BL * NV, FEAT),
            semantic=sem[b0:b0 + BL].reshape(BL * NS, SEM),
            patient=pat[b0:b0 + BL].reshape(BL * L, DM),
            **shared,
        ))
    res = bass_utils.run_bass_kernel_spmd(nc, in_maps, core_ids=list(range(NCORES)))
    ctx = np.concatenate([r["ctx"] for r in res.results], axis=0)
    v_attn = np.concatenate([r["v_attn"] for r in res.results], axis=0)
    s_attn = np.concatenate([r["s_attn"] for r in res.results], axis=0)
    return (ctx.astype(np.float32),
            v_attn.reshape(B, NV, 1).astype(np.float32),
            s_attn.reshape(B, NS, 1).astype(np.float32))
